# revision 24
# baseline (speedup 1.0000x reference)
"""Trainium2 Bass kernel for a 2-layer GraphSAGE classifier (BGNNClassifier).

Reference computation:
    h1 = relu(mean_agg(x) @ W1l.T + b1 + x @ W1r.T)
    h2 = relu(mean_agg(h1) @ W2l.T + b2 + h1 @ W2r.T)
    pooled = segment_mean(h2, batch)          # [G, H]
    out = log_softmax(pooled @ fcW.T + fcb)   # [G, O]

Distribution strategy (8 NeuronCores, SPMD, one NEFF):
  - Nodes are partitioned into 8 contiguous shards (12500 each); each core
    computes h1/h2 for its own nodes.  All dense math is bf16 (fp32 PSUM).
  - Layer 1 neighbor features are fully pre-arranged on the host: for each
    block of 4 destination tiles the host builds a packed table whose rows
    (one per SBUF partition) contain 64 lanes x 64 bf16 features, one lane
    per (tile, chunk).  The device just streams the table with plain
    contiguous DMA - no per-edge gather descriptors at all.
  - Layer 2 needs h1 of arbitrary nodes, so it uses dma_gather from the
    AllGather'd h1 table, stored as bf16 pair-rows [2 nodes x 64 = 256 B].
    Edges are grouped by (dst tile, table phase, src parity); each 256 B
    descriptor serves one edge (low or high half by parity).
  - Mean aggregation is computed on the TensorEngine as one-hot matmuls;
    the [128,128] one-hot-with-recip matrices are built on DVE from
    per-chunk metadata (bf16).  Per-graph pooling is another one-hot
    matmul; partial per-graph sums are AllReduce'd, then every core
    computes the (identical) logits + log_softmax.
"""

import numpy as np
import ml_dtypes

BF16 = ml_dtypes.bfloat16
FP8 = ml_dtypes.float8_e4m3

# ---------------------------------------------------------------------------
# Problem configuration
# ---------------------------------------------------------------------------
CFG = dict(
    N=100000,      # nodes
    E=1600000,     # edges
    D=64,          # in features
    H=64,          # hidden
    O=16,          # classes
    G=512,         # graphs
    NCORES=8,
    B=4,           # tiles per block
    CPP=4,         # chunk-cols per (tile, class)
    NCLS=4,        # L2 classes: (phase, parity)
    PACK2=500,     # per-(tile, class) edge budget while packing
    P=128,
)

_BUILD_CACHE = {}


# ---------------------------------------------------------------------------
# Host-side preprocessing
# ---------------------------------------------------------------------------

def _pack_tiles(counts, pack_limit, P=128):
    """Pack nodes (in order) into tiles s.t. every per-tile counter sum
    <= pack_limit and node count <= P.  counts: [n_nodes, K] int64.
    Returns list of tile start indices (len T+1, last == n_nodes)."""
    n = counts.shape[0]
    cum = np.concatenate([np.zeros((1, counts.shape[1]), np.int64),
                          np.cumsum(counts, axis=0)], axis=0)  # [n+1, K]
    starts = [0]
    s = 0
    while s < n:
        e_lim = min(n, s + P)
        e = e_lim
        for k in range(counts.shape[1]):
            ek = int(np.searchsorted(cum[:, k], cum[s, k] + pack_limit,
                                     side="right")) - 1
            e = min(e, ek)
        if e <= s:
            raise ValueError(
                f"node {s} alone exceeds pack limit (deg counts {counts[s]})")
        starts.append(e)
        s = e
    return starts



def _binpack_tiles(cnt, pack_limit, tile_cap, P=128):
    """Greedy FFD bin-packing of nodes into tiles.
    cnt: [n, K] per-node class counts.  Returns (t_of, q_of, n_tiles).
    Constraints per tile: <=P nodes, per-class sum <= pack_limit,
    total sum <= tile_cap."""
    n, K = cnt.shape
    tot = cnt.sum(axis=1)
    order = np.argsort(-tot, kind="stable")
    nbins = max(1, -(-n // P))
    bins_cnt = np.zeros((nbins, K), np.int64)
    bins_tot = np.zeros(nbins, np.int64)
    bins_n = np.zeros(nbins, np.int64)
    t_of = np.empty(n, np.int64)
    for u in order:
        c = cnt[u]
        fits = ((bins_cnt + c) <= pack_limit).all(axis=1)
        fits &= (bins_n < P) & (bins_tot + tot[u] <= tile_cap)
        idx = np.argmax(fits)
        if not fits[idx]:
            bins_cnt = np.concatenate(
                [bins_cnt, np.zeros((1, K), np.int64)], axis=0)
            bins_tot = np.concatenate([bins_tot, [0]])
            bins_n = np.concatenate([bins_n, [0]])
            idx = len(bins_n) - 1
        t_of[u] = idx
        bins_cnt[idx] += c
        bins_tot[idx] += tot[u]
        bins_n[idx] += 1
    # q within tile: order of assignment
    nb = len(bins_n)
    q_of = np.empty(n, np.int64)
    fill = np.zeros(nb, np.int64)
    for u in order:
        q_of[u] = fill[t_of[u]]
        fill[t_of[u]] += 1
    return t_of, q_of, nb


def _rank_within_groups(key, n_groups):
    """For int array key, return rank of each element within its key-group
    (stable order)."""
    order = np.argsort(key, kind="stable")
    sk = key[order]
    group_sizes = np.bincount(sk, minlength=n_groups)
    group_starts = np.concatenate([[0], np.cumsum(group_sizes)[:-1]])
    ranks_sorted = np.arange(len(key)) - group_starts[sk]
    ranks = np.empty(len(key), np.int64)
    ranks[order] = ranks_sorted
    return ranks


def _wrap_idx(idx_call):
    """dma_gather index layout: idx i -> [16r + i%16, i//16], replicated
    for the 8 Q7 cores.  idx_call: [n] int -> [128, n//16] int16."""
    n = idx_call.shape[0]
    assert n % 16 == 0
    w = idx_call.reshape(n // 16, 16).T.astype(np.int16)   # [16, n//16]
    return np.tile(w, (8, 1))                              # [128, n//16]


def preprocess(x, W1l, b1, W1r, W2l, b2, W2r, fcW, fcb, edge_index, batch,
               cfg=CFG):
    """Builds per-core input maps + layout info. Returns (in_maps, info)."""
    N, E, D, H, O, G = (cfg["N"], cfg["E"], cfg["D"], cfg["H"], cfg["O"],
                        cfg["G"])
    NC, B, CPP, NCLS, P = (cfg["NCORES"], cfg["B"], cfg["CPP"], cfg["NCLS"],
                           cfg["P"])
    PACK2 = cfg["PACK2"]
    NPC = N // NC
    NPH2 = 2                       # L2 table phases

    x = np.asarray(x, np.float32)
    src = np.asarray(edge_index[0], np.int64)
    dst = np.asarray(edge_index[1], np.int64)
    batch = np.asarray(batch, np.int64)

    deg = np.bincount(dst, minlength=N)
    recip = (1.0 / np.maximum(deg, 1)).astype(np.float32)
    gsize = np.bincount(batch, minlength=G)
    grecip_g = (1.0 / np.maximum(gsize, 1)).astype(np.float32)

    core_of = dst // NPC

    # --- iterative packing (L2 classes depend on packed positions) -------
    # initial h1row guess: contiguous packing, 128 nodes per tile
    u = np.arange(N)
    t_guess = (u % NPC) // P
    T0 = -(-NPC // P)
    h1row = (u // NPC) * (T0 * P) + t_guess * P + (u % NPC) % P
    PH_ROWS = NC * T0 * P // NPH2

    T = None
    t_of = q_of = None
    pack = PACK2
    for attempt in range(8):
        for _ in range(12):
            cls_e = (h1row[src] // PH_ROWS) * 2 + (h1row[src] & 1)  # 0..3
            cnt = np.bincount(dst * NCLS + cls_e, minlength=N * NCLS) \
                    .reshape(N, NCLS)
            starts_per_core = [
                _pack_tiles(cnt[k * NPC:(k + 1) * NPC], pack, P)
                for k in range(NC)]
            T_new = max(len(s) - 1 for s in starts_per_core)
            T_new = -(-T_new // B) * B
            t_of = np.empty(N, np.int64)
            q_of = np.empty(N, np.int64)
            for k in range(NC):
                st = np.asarray(starts_per_core[k])
                uu = np.arange(NPC)
                tk = np.searchsorted(st, uu, side="right") - 1
                t_of[k * NPC:(k + 1) * NPC] = tk
                q_of[k * NPC:(k + 1) * NPC] = uu - st[tk]
            h1row_new = (np.arange(N) // NPC) * (T_new * P) + t_of * P + q_of
            PH_ROWS_new = NC * T_new * P // NPH2
            assert PH_ROWS_new // 2 <= 32767, f"T={T_new} too big for idx"
            stable = (T == T_new and np.array_equal(h1row_new, h1row))
            h1row = h1row_new
            T = T_new
            PH_ROWS = PH_ROWS_new
            if stable:
                break
        # budget check with the final class assignment
        cls_e = (h1row[src] // PH_ROWS) * 2 + (h1row[src] & 1)
        tile_of_dst = t_of[dst]
        seg2 = (core_of * T + tile_of_dst) * NCLS + cls_e
        c2 = np.bincount(seg2, minlength=NC * T * NCLS)
        seg1 = core_of * T + tile_of_dst
        c1 = np.bincount(seg1, minlength=NC * T)
        if c2.max() <= CPP * P and c1.max() <= 16 * P:
            break
        pack -= 16
    else:
        raise ValueError(f"packing failed: L2 max {c2.max()}")
    NB = T // B
    NPH2 = 2
    PH_ROWS = NC * T * P // NPH2
    H1ROWS = NC * T * P

    # --- per-edge slot assignment ----------------------------------------
    # L1: rank within tile -> (ch, p)
    r1 = _rank_within_groups(seg1, NC * T)
    ch1 = r1 // P            # 0..15
    p1 = r1 % P
    # L2: rank within (tile, cls) -> (j, p)
    r2 = _rank_within_groups(seg2, NC * T * NCLS)
    j2 = r2 // P             # 0..CPP-1
    p2 = r2 % P
    pair_local = (h1row[src] % PH_ROWS) >> 1          # idx value
    ph_e = cls_e // 2
    par_e = cls_e & 1

    x_bf = x.astype(BF16)
    W1l = np.asarray(W1l, np.float32); W1r = np.asarray(W1r, np.float32)
    W2l = np.asarray(W2l, np.float32); W2r = np.asarray(W2r, np.float32)
    wts = np.concatenate([W1l.T, W1r.T, W2l.T, W2r.T], axis=1).astype(BF16)
    bias = np.stack([np.asarray(b1, np.float32),
                     np.asarray(b2, np.float32)], axis=1)       # [64, 2]
    fcw = np.concatenate([np.asarray(fcW, np.float32).T,
                          np.asarray(fcb, np.float32)[None, :]],
                         axis=0)                                # [65, 16]


    in_maps = []
    for k in range(NC):
        m = {}
        ek = np.nonzero(core_of == k)[0]
        tk = tile_of_dst[ek]

        # ---- L1 packed table [NB*128, 64*64] bf16 ----------------------
        tab1 = np.zeros((NB * P, 64, D), BF16)
        lane = (tk % B) * 16 + ch1[ek]
        rows = (tk // B) * P + p1[ek]
        tab1[rows, lane, :] = x_bf[src[ek]]
        m["tab1"] = tab1.reshape(NB * P, 64 * D)

        # ---- L2 idx tensor [128, NB*8*64] int16 ------------------------
        flat = np.zeros((NB, 64, P), np.int64)    # [block, gcol, p]
        gcol = ph_e[ek] * 32 + par_e[ek] * 16 + (tk % B) * 4 + j2[ek]
        flat[tk // B, gcol, p2[ek]] = pair_local[ek]
        cols = []
        for b in range(NB):
            for ph in range(NPH2):
                for c in range(4):
                    g0 = ph * 32 + c * 8
                    callidx = flat[b, g0:g0 + 8, :].reshape(-1)  # [1024]
                    cols.append(_wrap_idx(callidx))
        m["idxL2"] = np.concatenate(cols, axis=1)  # [128, NB*8*64]

        # ---- precomputed one-hot tensors (bf16) ------------------------
        qd = q_of[dst[ek]]
        rd = recip[dst[ek]].astype(BF16)
        oh1 = np.zeros((NB * P, 64 * P), BF16)
        lane1 = (tk % B) * 16 + ch1[ek]
        oh1[(tk // B) * P + p1[ek], lane1 * P + qd] = rd
        m["oh1"] = oh1
        oh2 = np.zeros((NB * P, 64 * P), BF16)
        ch2 = cls_e[ek] * 4 + j2[ek]
        bc2 = (tk % B) * 16 + ch2
        oh2[(tk // B) * P + p2[ek], bc2 * P + qd] = rd
        m["oh2"] = oh2
        ghoh = np.zeros((NB * P, B * G), BF16)
        nodes = np.arange(k * NPC, (k + 1) * NPC)
        tn = t_of[nodes]
        ghoh[(tn // B) * P + q_of[nodes], (tn % B) * G + batch[nodes]] = \
            grecip_g[batch[nodes]].astype(BF16)
        m["ghoh"] = ghoh

        # ---- xT slab [64, T*128] bf16 ---------------------------------
        xT = np.zeros((D, T * P), np.float32)
        xT[:, t_of[nodes] * P + q_of[nodes]] = x[nodes].T
        m["xT"] = xT.astype(BF16)

        m["wts"] = wts
        m["bias"] = bias
        m["fcw"] = fcw
        in_maps.append(m)

    info = dict(T=T, NB=NB, PH_ROWS=PH_ROWS, H1ROWS=H1ROWS,
                h1row=h1row, t_of=t_of, q_of=q_of)
    return in_maps, info


# ---------------------------------------------------------------------------
# Numpy simulation of the device algorithm (validates host prep/layout)
# ---------------------------------------------------------------------------

def device_sim(in_maps, info, cfg=CFG):
    N, D, H, O, G = cfg["N"], cfg["D"], cfg["H"], cfg["O"], cfg["G"]
    NC, B, P = cfg["NCORES"], cfg["B"], cfg["P"]
    T, NB = info["T"], info["NB"]
    PH_ROWS = info["PH_ROWS"]
    f32 = lambda a: np.asarray(a, np.float32)
    bf = lambda a: np.asarray(a, BF16).astype(np.float32)

    h1T_all = []
    # ---- layer 1 --------------------------------------------------------
    for k in range(NC):
        m = in_maps[k]
        wts = f32(m["wts"]); xT = f32(m["xT"])
        tab1 = f32(m["tab1"]).reshape(NB * P, 64, D)
        oh1 = f32(m["oh1"])
        h1T = np.zeros((H, T * P), np.float32)
        for b in range(NB):
            g1 = tab1[b * P:(b + 1) * P]           # [128, 64, 64]
            oh1b = oh1[b * P:(b + 1) * P]          # [128, 64*128]
            for ti in range(B):
                t = b * B + ti
                aggrT = np.zeros((D, P), np.float32)
                for ch in range(16):
                    bc = ti * 16 + ch
                    oh = oh1b[:, bc * P:(bc + 1) * P]
                    aggrT += g1[:, bc, :].T @ oh
                aggrT = bf(aggrT)
                pre = (wts[:, 0:64].T @ aggrT
                       + wts[:, 64:128].T @ xT[:, t * P:(t + 1) * P]
                       + f32(m["bias"])[:, 0][:, None])
                h1T[:, t * P:(t + 1) * P] = bf(np.maximum(pre, 0.0))
        h1T_all.append(h1T)

    # allgather h1 into pair-rows [H1ROWS//2, 128]
    h1_full = np.concatenate([h1T.T for h1T in h1T_all], axis=0)  # [H1ROWS, 64]
    h1_pairs = bf(h1_full).reshape(-1, 2 * D)       # [H1ROWS//2, 128]

    pooled = np.zeros((D, G), np.float32)
    # ---- layer 2 --------------------------------------------------------
    for k in range(NC):
        m = in_maps[k]
        wts = f32(m["wts"])
        oh2 = f32(m["oh2"]); ghohm = f32(m["ghoh"])
        h1T = h1T_all[k]
        idxw = m["idxL2"]
        for b in range(NB):
            # unwrap the 8 calls of this block
            g2 = np.zeros((P, 64, 2 * D), np.float32)   # [p, gcol, 128]
            for ph in range(2):
                base_row = ph * PH_ROWS // 2
                for c in range(4):
                    cw = (b * 8 + ph * 4 + c) * 64
                    blk = idxw[:16, cw:cw + 64]
                    idxs = blk.T.reshape(-1).astype(np.int64)   # [1024]
                    rows = h1_pairs[base_row + idxs]            # [1024, 128]
                    g0 = ph * 32 + c * 8
                    g2[:, g0:g0 + 8, :] = \
                        rows.reshape(8, P, 2 * D).transpose(1, 0, 2)
            oh2b = oh2[b * P:(b + 1) * P]
            ghb = ghohm[b * P:(b + 1) * P]
            for ti in range(B):
                t = b * B + ti
                aggrT = np.zeros((D, P), np.float32)
                for cls in range(4):
                    phh, par = cls // 2, cls % 2
                    for j in range(4):
                        bc = ti * 16 + cls * 4 + j
                        gcol = phh * 32 + par * 16 + ti * 4 + j
                        oh = oh2b[:, bc * P:(bc + 1) * P]
                        aggrT += g2[:, gcol, par * D:(par + 1) * D].T @ oh
                aggrT = bf(aggrT)
                pre = (wts[:, 128:192].T @ aggrT
                       + wts[:, 192:256].T @ h1T[:, t * P:(t + 1) * P]
                       + f32(m["bias"])[:, 1][:, None])
                h2T = bf(np.maximum(pre, 0.0))
                ghoh = ghb[:, ti * G:(ti + 1) * G]
                pooled += bf(h2T) @ ghoh

    m0 = in_maps[0]
    poolA = np.concatenate([pooled, np.ones((1, G), np.float32)], axis=0)
    logits = poolA.T @ f32(m0["fcw"])
    mx = logits.max(axis=1, keepdims=True)
    lse = np.log(np.exp(logits - mx).sum(axis=1, keepdims=True))
    return logits - mx - lse


# ---------------------------------------------------------------------------
# Bass kernel builder
# ---------------------------------------------------------------------------

def build_kernel(T, cfg=CFG):
    import concourse.bass as bass
    import concourse.bacc as bacc
    import concourse.tile as tile
    import concourse.mybir as mybir
    from concourse.masks import make_identity

    F32 = mybir.dt.float32
    BF = mybir.dt.bfloat16
    F8 = mybir.dt.float8e4
    I16 = mybir.dt.int16
    AF = mybir.ActivationFunctionType
    OP = mybir.AluOpType

    N, D, H, O, G = cfg["N"], cfg["E"], cfg["D"], cfg["H"], cfg["G"]
    N, E, D, H, O, G = (cfg["N"], cfg["E"], cfg["D"], cfg["H"], cfg["O"],
                        cfg["G"])
    NC, B, P = cfg["NCORES"], cfg["B"], cfg["P"]
    NB = T // B
    NPH2 = 2
    PH_ROWS = NC * T * P // NPH2
    H1ROWS = NC * T * P

    nc = bacc.Bacc("TRN2", target_bir_lowering=False, debug=False,
                   num_devices=NC, num_swdge_queues=4)

    oh1_t = nc.dram_tensor("oh1", [NB * P, 64 * P], BF, kind="ExternalInput")
    oh2_t = nc.dram_tensor("oh2", [NB * P, 64 * P], BF, kind="ExternalInput")
    ghoh_t = nc.dram_tensor("ghoh", [NB * P, B * G], BF,
                            kind="ExternalInput")
    tab1_t = nc.dram_tensor("tab1", [NB * P, 64 * D], BF,
                            kind="ExternalInput")
    idxL2_t = nc.dram_tensor("idxL2", [P, NB * 8 * 64], I16,
                             kind="ExternalInput")
    xT_t = nc.dram_tensor("xT", [D, T * P], BF, kind="ExternalInput")
    wts_t = nc.dram_tensor("wts", [D, 256], BF, kind="ExternalInput")
    bias_t = nc.dram_tensor("bias", [D, 2], F32, kind="ExternalInput")
    fcw_t = nc.dram_tensor("fcw", [D + 1, O], F32, kind="ExternalInput")
    out_t = nc.dram_tensor("out", [G, O], F32, kind="ExternalOutput")

    with tile.TileContext(nc, num_cores=NC) as tc:
        with (
            tc.tile_pool(name="cst", bufs=1) as cst,
            tc.tile_pool(name="slab", bufs=1) as slab,
            tc.tile_pool(name="g1p", bufs=3) as g1p,
            tc.tile_pool(name="g2p", bufs=2) as g2p,
            tc.tile_pool(name="idxp", bufs=2) as idxp,
            tc.tile_pool(name="ohbp", bufs=3) as ohbp,
            tc.tile_pool(name="ghp", bufs=3) as ghp,
            tc.tile_pool(name="agp", bufs=2) as agp,
            tc.tile_pool(name="rowp", bufs=3) as rowp,
            tc.tile_pool(name="smallp", bufs=2) as smallp,
            tc.tile_pool(name="ps_ag", bufs=2, space="PSUM") as ps_ag,
            tc.tile_pool(name="ps_h", bufs=2, space="PSUM") as ps_h,
            tc.tile_pool(name="ps_tr", bufs=2, space="PSUM") as ps_tr,
            tc.tile_pool(name="ps_pool", bufs=1, space="PSUM") as ps_pool,
            tc.tile_pool(name="dram", bufs=1, space="DRAM") as dram,
        ):
            # ---- constants -------------------------------------------------
            wts = cst.tile([D, 256], BF)
            nc.sync.dma_start(wts[:], wts_t[:])
            bias = cst.tile([D, 2], F32)
            nc.sync.dma_start(bias[:], bias_t[:])
            fcw = cst.tile([D + 1, O], F32)
            nc.sync.dma_start(fcw[:], fcw_t[:])
            xT = slab.tile([D, T * P], BF)
            nc.sync.dma_start(xT[:], xT_t[:])
            identb = cst.tile([P, P], BF)
            make_identity(nc, identb[:])
            # preload ACT Exp/Ln tables so the tail doesn't stall on them
            warm = cst.tile([1, 4], F32)
            nc.gpsimd.memset(warm[:], 1.0)
            warm2 = cst.tile([1, 4], F32)
            nc.scalar.activation(out=warm2[:], in_=warm[:], func=AF.Exp)
            nc.scalar.activation(out=warm2[:], in_=warm[:], func=AF.Ln)

            h1T = slab.tile([D, T * P], BF)

            h1_local = dram.tile([T * P, D], BF)
            h1_full = dram.tile([H1ROWS // 2, 2 * D], BF, addr_space="Shared")
            pool_in = dram.tile([D, G], F32)
            pool_out = dram.tile([D, G], F32, addr_space="Shared")

            psum_pool = ps_pool.tile([D, G], F32)

            def tile_tail(lyr, t, aggrT, ti, ghoh_sb):
                """weights + relu (+transpose/pool for L2)."""
                wcol = 0 if lyr == 0 else 128
                psum_hT = ps_h.tile([D, P], F32, tag="hT")
                nc.tensor.matmul(
                    psum_hT[:], lhsT=wts[:, wcol:wcol + 64],
                    rhs=aggrT[:, ti * P:(ti + 1) * P],
                    start=True, stop=False)
                rhs_self = (xT if lyr == 0 else h1T)
                nc.tensor.matmul(
                    psum_hT[:], lhsT=wts[:, wcol + 64:wcol + 128],
                    rhs=rhs_self[:, t * P:(t + 1) * P],
                    start=False, stop=True)
                if lyr == 0:
                    nc.scalar.activation(
                        out=h1T[:, t * P:(t + 1) * P], in_=psum_hT[:],
                        func=AF.Relu, bias=bias[:, 0:1], scale=1.0)
                    psum_tr = ps_tr.tile([P, D], BF, tag="tr")
                    nc.tensor.transpose(
                        psum_tr[:], h1T[:, t * P:(t + 1) * P],
                        identb[:D, :D])
                    h1row = rowp.tile([P, D], BF, tag="row")
                    nc.scalar.activation(out=h1row[:], in_=psum_tr[:],
                                         func=AF.Copy, scale=1.0)
                    nc.sync.dma_start(
                        h1_local[:][t * P:(t + 1) * P, :], h1row[:])
                else:
                    h2T = smallp.tile([D, P], BF, tag="h2T")
                    nc.scalar.activation(
                        out=h2T[:], in_=psum_hT[:],
                        func=AF.Relu, bias=bias[:, 1:2], scale=1.0)
                    psum_tr = ps_tr.tile([P, D], BF, tag="tr")
                    nc.tensor.transpose(psum_tr[:], h2T[:], identb[:D, :D])
                    h2row = rowp.tile([P, D], BF, tag="row")
                    nc.scalar.activation(out=h2row[:], in_=psum_tr[:],
                                         func=AF.Copy, scale=1.0)
                    nc.tensor.matmul(
                        psum_pool[:], lhsT=h2row[:],
                        rhs=ghoh_sb[:, ti * G:(ti + 1) * G],
                        start=(t == 0), stop=(t == T - 1),
                        skip_group_check=True,
                    )

            # ================= layer 1 =====================================
            for b in range(NB):
                g1 = g1p.tile([P, 64 * D], BF, tag="g1")
                nc.sync.dma_start(g1[:], tab1_t[b * P:(b + 1) * P, :])
                oh_sb = ohbp.tile([P, 64 * P], BF, tag="ohblk")
                nc.sync.dma_start(oh_sb[:], oh1_t[b * P:(b + 1) * P, :])
                psum_bank = ps_ag.tile([D, B * P], F32, tag="aggr")
                for ti in range(B):
                    t = b * B + ti
                    for ch in range(16):
                        lane = ti * 16 + ch
                        nc.tensor.matmul(
                            psum_bank[:, ti * P:(ti + 1) * P],
                            lhsT=g1[:, lane * D:(lane + 1) * D],
                            rhs=oh_sb[:, lane * P:(lane + 1) * P],
                            start=(ch == 0), stop=(ch == 15),
                        )
                aggrT = agp.tile([D, B * P], BF, tag="aggrT")
                nc.scalar.activation(out=aggrT[:], in_=psum_bank[:],
                                     func=AF.Copy, scale=1.0)
                for ti in range(B):
                    tile_tail(0, b * B + ti, aggrT, ti, None)

            # ================= AllGather h1 ================================
            nc.gpsimd.collective_compute(
                "AllGather", mybir.AluOpType.bypass,
                replica_groups=[list(range(NC))],
                ins=[h1_local.opt()], outs=[h1_full.opt()],
            )

            # ================= layer 2 =====================================
            for b in range(NB):
                idx_sb = idxp.tile([P, 8 * 64], I16, tag="idx")
                nc.sync.dma_start(
                    idx_sb[:], idxL2_t[:, b * 8 * 64:(b + 1) * 8 * 64])
                g2 = g2p.tile([P, 64, 2 * D], BF, tag="g2")
                for ph in range(NPH2):
                    lo = ph * (PH_ROWS // 2)
                    in_ap = h1_full[:][lo:lo + PH_ROWS // 2, :]
                    for c in range(4):
                        g0 = ph * 32 + c * 8
                        nc.gpsimd.dma_gather(
                            out_ap=g2[:, g0:g0 + 8, :],
                            in_ap=in_ap,
                            idxs_ap=idx_sb[:, (ph * 4 + c) * 64:
                                           (ph * 4 + c + 1) * 64],
                            num_idxs=1024,
                            num_idxs_reg=1024,
                            elem_size=2 * D,
                            single_packet=True,
                            queue_num=(b * 8 + ph * 4 + c) % 4,
                        )
                oh_sb = ohbp.tile([P, 64 * P], BF, tag="ohblk")
                nc.sync.dma_start(oh_sb[:], oh2_t[b * P:(b + 1) * P, :])
                ghoh_sb = ghp.tile([P, B * G], BF, tag="gh")
                nc.sync.dma_start(ghoh_sb[:], ghoh_t[b * P:(b + 1) * P, :])
                psum_bank = ps_ag.tile([D, B * P], F32, tag="aggr")
                for ti in range(B):
                    t = b * B + ti
                    nmm = 0
                    for cls in range(4):
                        phh, par = cls // 2, cls % 2
                        for j in range(4):
                            bc = ti * 16 + cls * 4 + j
                            gcol = phh * 32 + par * 16 + ti * 4 + j
                            nc.tensor.matmul(
                                psum_bank[:, ti * P:(ti + 1) * P],
                                lhsT=g2[:, gcol, par * D:(par + 1) * D],
                                rhs=oh_sb[:, bc * P:(bc + 1) * P],
                                start=(nmm == 0), stop=(nmm == 15),
                            )
                            nmm += 1
                aggrT = agp.tile([D, B * P], BF, tag="aggrT")
                nc.scalar.activation(out=aggrT[:], in_=psum_bank[:],
                                     func=AF.Copy, scale=1.0)
                for ti in range(B):
                    tile_tail(1, b * B + ti, aggrT, ti, ghoh_sb)

            # ---- pooled AllReduce + logits + log_softmax -------------------
            pooled_sb = slab.tile([D + 1, G], F32)
            nc.gpsimd.memset(pooled_sb[D:D + 1, :], 1.0)
            nc.vector.tensor_copy(out=pooled_sb[:D, :], in_=psum_pool[:])
            nc.sync.dma_start(pool_in[:], pooled_sb[:D, :])
            nc.gpsimd.collective_compute(
                "AllReduce", mybir.AluOpType.add,
                replica_groups=[list(range(NC))],
                ins=[pool_in.opt()], outs=[pool_out.opt()],
            )
            nc.sync.dma_start(pooled_sb[:D, :], pool_out[:])
            for gt in range(-(-G // P)):
                gsz = min(P, G - gt * P)
                psum_lg = ps_pool.tile([gsz, O], F32, tag="lg")
                nc.tensor.matmul(
                    psum_lg[:], lhsT=pooled_sb[:, gt * P:gt * P + gsz],
                    rhs=fcw[:], start=True, stop=True)
                mx = smallp.tile([gsz, 1], F32, tag="mx")
                nc.vector.tensor_reduce(
                    out=mx[:], in_=psum_lg[:], axis=mybir.AxisListType.X,
                    op=OP.max)
                nmx = smallp.tile([gsz, 1], F32, tag="nmx")
                nc.vector.tensor_scalar(
                    out=nmx[:], in0=mx[:], scalar1=-1.0, scalar2=None,
                    op0=OP.mult)
                ex = smallp.tile([gsz, O], F32, tag="ex")
                sumexp = smallp.tile([gsz, 1], F32, tag="se")
                nc.scalar.activation(
                    out=ex[:], in_=psum_lg[:], func=AF.Exp,
                    bias=nmx[:], scale=1.0, accum_out=sumexp[:])
                lse = smallp.tile([gsz, 1], F32, tag="lse")
                nc.scalar.activation(
                    out=lse[:], in_=sumexp[:], func=AF.Ln)
                res = smallp.tile([gsz, O], F32, tag="res")
                nc.vector.tensor_scalar(
                    out=res[:], in0=psum_lg[:], scalar1=nmx[:],
                    scalar2=lse[:], op0=OP.add, op1=OP.subtract)
                nc.sync.dma_start(out_t[gt * P:gt * P + gsz, :], res[:])

    nc.compile()
    return nc


# ---------------------------------------------------------------------------
# Entry point
# ---------------------------------------------------------------------------

def kernel(x, W1l, b1, W1r, W2l, b2, W2r, fcW, fcb, edge_index, batch,
           _cfg=None, _collect=None):
    cfg = _cfg or CFG
    in_maps, info = preprocess(x, W1l, b1, W1r, W2l, b2, W2r, fcW, fcb,
                               edge_index, batch, cfg)
    key = (info["T"], tuple(sorted(cfg.items())))
    if key not in _BUILD_CACHE:
        _BUILD_CACHE[key] = build_kernel(info["T"], cfg)
    nc = _BUILD_CACHE[key]

    from concourse.bass_utils import run_bass_kernel_spmd
    res = run_bass_kernel_spmd(
        nc, in_maps, core_ids=list(range(cfg["NCORES"])),
        **(_collect or {}))
    if _collect is not None:
        kernel._last_result = res
    return res.results[0]["out"]


if __name__ == "__main__":
    pass


# revision 25
# speedup vs baseline: 1.0017x; 1.0017x over previous
"""Trainium2 Bass kernel for a 2-layer GraphSAGE classifier (BGNNClassifier).

Reference computation:
    h1 = relu(mean_agg(x) @ W1l.T + b1 + x @ W1r.T)
    h2 = relu(mean_agg(h1) @ W2l.T + b2 + h1 @ W2r.T)
    pooled = segment_mean(h2, batch)          # [G, H]
    out = log_softmax(pooled @ fcW.T + fcb)   # [G, O]

Distribution strategy (8 NeuronCores, SPMD, one NEFF):
  - Nodes are partitioned into 8 contiguous shards (12500 each); each core
    computes h1/h2 for its own nodes.  All dense math is bf16 (fp32 PSUM).
  - Layer 1 neighbor features are fully pre-arranged on the host: for each
    block of 4 destination tiles the host builds a packed table whose rows
    (one per SBUF partition) contain 64 lanes x 64 bf16 features, one lane
    per (tile, chunk).  The device just streams the table with plain
    contiguous DMA - no per-edge gather descriptors at all.
  - Layer 2 needs h1 of arbitrary nodes, so it uses dma_gather from the
    AllGather'd h1 table, stored as bf16 pair-rows [2 nodes x 64 = 256 B].
    Edges are grouped by (dst tile, table phase, src parity); each 256 B
    descriptor serves one edge (low or high half by parity).
  - Mean aggregation is computed on the TensorEngine as one-hot matmuls;
    the [128,128] one-hot-with-recip matrices are built on DVE from
    per-chunk metadata (bf16).  Per-graph pooling is another one-hot
    matmul; partial per-graph sums are AllReduce'd, then every core
    computes the (identical) logits + log_softmax.
"""

import numpy as np
import ml_dtypes

BF16 = ml_dtypes.bfloat16
FP8 = ml_dtypes.float8_e4m3

# ---------------------------------------------------------------------------
# Problem configuration
# ---------------------------------------------------------------------------
CFG = dict(
    N=100000,      # nodes
    E=1600000,     # edges
    D=64,          # in features
    H=64,          # hidden
    O=16,          # classes
    G=512,         # graphs
    NCORES=8,
    B=4,           # tiles per block
    CPP=4,         # chunk-cols per (tile, class)
    NCLS=4,        # L2 classes: (phase, parity)
    PACK2=500,     # per-(tile, class) edge budget while packing
    P=128,
)

_BUILD_CACHE = {}


# ---------------------------------------------------------------------------
# Host-side preprocessing
# ---------------------------------------------------------------------------

def _pack_tiles(counts, pack_limit, P=128):
    """Pack nodes (in order) into tiles s.t. every per-tile counter sum
    <= pack_limit and node count <= P.  counts: [n_nodes, K] int64.
    Returns list of tile start indices (len T+1, last == n_nodes)."""
    n = counts.shape[0]
    cum = np.concatenate([np.zeros((1, counts.shape[1]), np.int64),
                          np.cumsum(counts, axis=0)], axis=0)  # [n+1, K]
    starts = [0]
    s = 0
    while s < n:
        e_lim = min(n, s + P)
        e = e_lim
        for k in range(counts.shape[1]):
            ek = int(np.searchsorted(cum[:, k], cum[s, k] + pack_limit,
                                     side="right")) - 1
            e = min(e, ek)
        if e <= s:
            raise ValueError(
                f"node {s} alone exceeds pack limit (deg counts {counts[s]})")
        starts.append(e)
        s = e
    return starts



def _binpack_tiles(cnt, pack_limit, tile_cap, P=128):
    """Greedy FFD bin-packing of nodes into tiles.
    cnt: [n, K] per-node class counts.  Returns (t_of, q_of, n_tiles).
    Constraints per tile: <=P nodes, per-class sum <= pack_limit,
    total sum <= tile_cap."""
    n, K = cnt.shape
    tot = cnt.sum(axis=1)
    order = np.argsort(-tot, kind="stable")
    nbins = max(1, -(-n // P))
    bins_cnt = np.zeros((nbins, K), np.int64)
    bins_tot = np.zeros(nbins, np.int64)
    bins_n = np.zeros(nbins, np.int64)
    t_of = np.empty(n, np.int64)
    for u in order:
        c = cnt[u]
        fits = ((bins_cnt + c) <= pack_limit).all(axis=1)
        fits &= (bins_n < P) & (bins_tot + tot[u] <= tile_cap)
        idx = np.argmax(fits)
        if not fits[idx]:
            bins_cnt = np.concatenate(
                [bins_cnt, np.zeros((1, K), np.int64)], axis=0)
            bins_tot = np.concatenate([bins_tot, [0]])
            bins_n = np.concatenate([bins_n, [0]])
            idx = len(bins_n) - 1
        t_of[u] = idx
        bins_cnt[idx] += c
        bins_tot[idx] += tot[u]
        bins_n[idx] += 1
    # q within tile: order of assignment
    nb = len(bins_n)
    q_of = np.empty(n, np.int64)
    fill = np.zeros(nb, np.int64)
    for u in order:
        q_of[u] = fill[t_of[u]]
        fill[t_of[u]] += 1
    return t_of, q_of, nb


def _rank_within_groups(key, n_groups):
    """For int array key, return rank of each element within its key-group
    (stable order)."""
    order = np.argsort(key, kind="stable")
    sk = key[order]
    group_sizes = np.bincount(sk, minlength=n_groups)
    group_starts = np.concatenate([[0], np.cumsum(group_sizes)[:-1]])
    ranks_sorted = np.arange(len(key)) - group_starts[sk]
    ranks = np.empty(len(key), np.int64)
    ranks[order] = ranks_sorted
    return ranks


def _wrap_idx(idx_call):
    """dma_gather index layout: idx i -> [16r + i%16, i//16], replicated
    for the 8 Q7 cores.  idx_call: [n] int -> [128, n//16] int16."""
    n = idx_call.shape[0]
    assert n % 16 == 0
    w = idx_call.reshape(n // 16, 16).T.astype(np.int16)   # [16, n//16]
    return np.tile(w, (8, 1))                              # [128, n//16]


def preprocess(x, W1l, b1, W1r, W2l, b2, W2r, fcW, fcb, edge_index, batch,
               cfg=CFG):
    """Builds per-core input maps + layout info. Returns (in_maps, info)."""
    N, E, D, H, O, G = (cfg["N"], cfg["E"], cfg["D"], cfg["H"], cfg["O"],
                        cfg["G"])
    NC, B, CPP, NCLS, P = (cfg["NCORES"], cfg["B"], cfg["CPP"], cfg["NCLS"],
                           cfg["P"])
    PACK2 = cfg["PACK2"]
    NPC = N // NC
    NPH2 = 2                       # L2 table phases

    x = np.asarray(x, np.float32)
    src = np.asarray(edge_index[0], np.int64)
    dst = np.asarray(edge_index[1], np.int64)
    batch = np.asarray(batch, np.int64)

    deg = np.bincount(dst, minlength=N)
    recip = (1.0 / np.maximum(deg, 1)).astype(np.float32)
    gsize = np.bincount(batch, minlength=G)
    grecip_g = (1.0 / np.maximum(gsize, 1)).astype(np.float32)

    core_of = dst // NPC

    # --- iterative packing (L2 classes depend on packed positions) -------
    # initial h1row guess: contiguous packing, 128 nodes per tile
    u = np.arange(N)
    t_guess = (u % NPC) // P
    T0 = -(-NPC // P)
    h1row = (u // NPC) * (T0 * P) + t_guess * P + (u % NPC) % P
    PH_ROWS = NC * T0 * P // NPH2

    T = None
    t_of = q_of = None
    pack = PACK2
    for attempt in range(8):
        for _ in range(12):
            cls_e = (h1row[src] // PH_ROWS) * 2 + (h1row[src] & 1)  # 0..3
            cnt = np.bincount(dst * NCLS + cls_e, minlength=N * NCLS) \
                    .reshape(N, NCLS)
            starts_per_core = [
                _pack_tiles(cnt[k * NPC:(k + 1) * NPC], pack, P)
                for k in range(NC)]
            T_new = max(len(s) - 1 for s in starts_per_core)
            T_new = -(-T_new // B) * B
            t_of = np.empty(N, np.int64)
            q_of = np.empty(N, np.int64)
            for k in range(NC):
                st = np.asarray(starts_per_core[k])
                uu = np.arange(NPC)
                tk = np.searchsorted(st, uu, side="right") - 1
                t_of[k * NPC:(k + 1) * NPC] = tk
                q_of[k * NPC:(k + 1) * NPC] = uu - st[tk]
            h1row_new = (np.arange(N) // NPC) * (T_new * P) + t_of * P + q_of
            PH_ROWS_new = NC * T_new * P // NPH2
            assert PH_ROWS_new // 2 <= 32767, f"T={T_new} too big for idx"
            stable = (T == T_new and np.array_equal(h1row_new, h1row))
            h1row = h1row_new
            T = T_new
            PH_ROWS = PH_ROWS_new
            if stable:
                break
        # budget check with the final class assignment
        cls_e = (h1row[src] // PH_ROWS) * 2 + (h1row[src] & 1)
        tile_of_dst = t_of[dst]
        seg2 = (core_of * T + tile_of_dst) * NCLS + cls_e
        c2 = np.bincount(seg2, minlength=NC * T * NCLS)
        seg1 = core_of * T + tile_of_dst
        c1 = np.bincount(seg1, minlength=NC * T)
        if c2.max() <= CPP * P and c1.max() <= 16 * P:
            break
        pack -= 16
    else:
        raise ValueError(f"packing failed: L2 max {c2.max()}")
    NB = T // B
    NPH2 = 2
    PH_ROWS = NC * T * P // NPH2
    H1ROWS = NC * T * P

    # --- per-edge slot assignment ----------------------------------------
    # L1: rank within tile -> (ch, p)
    r1 = _rank_within_groups(seg1, NC * T)
    ch1 = r1 // P            # 0..15
    p1 = r1 % P
    # L2: rank within (tile, cls) -> (j, p)
    r2 = _rank_within_groups(seg2, NC * T * NCLS)
    j2 = r2 // P             # 0..CPP-1
    p2 = r2 % P
    pair_local = (h1row[src] % PH_ROWS) >> 1          # idx value
    ph_e = cls_e // 2
    par_e = cls_e & 1

    x_bf = x.astype(BF16)
    W1l = np.asarray(W1l, np.float32); W1r = np.asarray(W1r, np.float32)
    W2l = np.asarray(W2l, np.float32); W2r = np.asarray(W2r, np.float32)
    wts = np.concatenate([W1l.T, W1r.T, W2l.T, W2r.T], axis=1).astype(BF16)
    bias = np.stack([np.asarray(b1, np.float32),
                     np.asarray(b2, np.float32)], axis=1)       # [64, 2]
    fcw = np.concatenate([np.asarray(fcW, np.float32).T,
                          np.asarray(fcb, np.float32)[None, :]],
                         axis=0)                                # [65, 16]


    in_maps = []
    for k in range(NC):
        m = {}
        ek = np.nonzero(core_of == k)[0]
        tk = tile_of_dst[ek]

        # ---- L1 packed table [NB*128, 64*64] bf16 ----------------------
        tab1 = np.zeros((NB * P, 64, D), BF16)
        lane = (tk % B) * 16 + ch1[ek]
        rows = (tk // B) * P + p1[ek]
        tab1[rows, lane, :] = x_bf[src[ek]]
        m["tab1"] = tab1.reshape(NB * P, 64 * D)

        # ---- L2 idx tensor [128, NB*8*64] int16 ------------------------
        flat = np.zeros((NB, 64, P), np.int64)    # [block, gcol, p]
        gcol = ph_e[ek] * 32 + par_e[ek] * 16 + (tk % B) * 4 + j2[ek]
        flat[tk // B, gcol, p2[ek]] = pair_local[ek]
        cols = []
        for b in range(NB):
            for ph in range(NPH2):
                for c in range(4):
                    g0 = ph * 32 + c * 8
                    callidx = flat[b, g0:g0 + 8, :].reshape(-1)  # [1024]
                    cols.append(_wrap_idx(callidx))
        m["idxL2"] = np.concatenate(cols, axis=1)  # [128, NB*8*64]

        # ---- precomputed one-hot tensors (bf16) ------------------------
        qd = q_of[dst[ek]]
        rd = recip[dst[ek]].astype(BF16)
        oh1 = np.zeros((NB * P, 64 * P), BF16)
        lane1 = (tk % B) * 16 + ch1[ek]
        oh1[(tk // B) * P + p1[ek], lane1 * P + qd] = rd
        m["oh1"] = oh1
        oh2 = np.zeros((NB * P, 64 * P), BF16)
        ch2 = cls_e[ek] * 4 + j2[ek]
        bc2 = (tk % B) * 16 + ch2
        oh2[(tk // B) * P + p2[ek], bc2 * P + qd] = rd
        m["oh2"] = oh2
        ghoh = np.zeros((NB * P, B * G), BF16)
        nodes = np.arange(k * NPC, (k + 1) * NPC)
        tn = t_of[nodes]
        ghoh[(tn // B) * P + q_of[nodes], (tn % B) * G + batch[nodes]] = \
            grecip_g[batch[nodes]].astype(BF16)
        m["ghoh"] = ghoh

        # ---- xT slab [64, T*128] bf16 ---------------------------------
        xT = np.zeros((D, T * P), np.float32)
        xT[:, t_of[nodes] * P + q_of[nodes]] = x[nodes].T
        m["xT"] = xT.astype(BF16)

        m["wts"] = wts
        m["bias"] = bias
        m["fcw"] = fcw
        in_maps.append(m)

    info = dict(T=T, NB=NB, PH_ROWS=PH_ROWS, H1ROWS=H1ROWS,
                h1row=h1row, t_of=t_of, q_of=q_of)
    return in_maps, info


# ---------------------------------------------------------------------------
# Numpy simulation of the device algorithm (validates host prep/layout)
# ---------------------------------------------------------------------------

def device_sim(in_maps, info, cfg=CFG):
    N, D, H, O, G = cfg["N"], cfg["D"], cfg["H"], cfg["O"], cfg["G"]
    NC, B, P = cfg["NCORES"], cfg["B"], cfg["P"]
    T, NB = info["T"], info["NB"]
    PH_ROWS = info["PH_ROWS"]
    f32 = lambda a: np.asarray(a, np.float32)
    bf = lambda a: np.asarray(a, BF16).astype(np.float32)

    h1T_all = []
    # ---- layer 1 --------------------------------------------------------
    for k in range(NC):
        m = in_maps[k]
        wts = f32(m["wts"]); xT = f32(m["xT"])
        tab1 = f32(m["tab1"]).reshape(NB * P, 64, D)
        oh1 = f32(m["oh1"])
        h1T = np.zeros((H, T * P), np.float32)
        for b in range(NB):
            g1 = tab1[b * P:(b + 1) * P]           # [128, 64, 64]
            oh1b = oh1[b * P:(b + 1) * P]          # [128, 64*128]
            for ti in range(B):
                t = b * B + ti
                aggrT = np.zeros((D, P), np.float32)
                for ch in range(16):
                    bc = ti * 16 + ch
                    oh = oh1b[:, bc * P:(bc + 1) * P]
                    aggrT += g1[:, bc, :].T @ oh
                aggrT = bf(aggrT)
                pre = (wts[:, 0:64].T @ aggrT
                       + wts[:, 64:128].T @ xT[:, t * P:(t + 1) * P]
                       + f32(m["bias"])[:, 0][:, None])
                h1T[:, t * P:(t + 1) * P] = bf(np.maximum(pre, 0.0))
        h1T_all.append(h1T)

    # allgather h1 into pair-rows [H1ROWS//2, 128]
    h1_full = np.concatenate([h1T.T for h1T in h1T_all], axis=0)  # [H1ROWS, 64]
    h1_pairs = bf(h1_full).reshape(-1, 2 * D)       # [H1ROWS//2, 128]

    pooled = np.zeros((D, G), np.float32)
    # ---- layer 2 --------------------------------------------------------
    for k in range(NC):
        m = in_maps[k]
        wts = f32(m["wts"])
        oh2 = f32(m["oh2"]); ghohm = f32(m["ghoh"])
        h1T = h1T_all[k]
        idxw = m["idxL2"]
        for b in range(NB):
            # unwrap the 8 calls of this block
            g2 = np.zeros((P, 64, 2 * D), np.float32)   # [p, gcol, 128]
            for ph in range(2):
                base_row = ph * PH_ROWS // 2
                for c in range(4):
                    cw = (b * 8 + ph * 4 + c) * 64
                    blk = idxw[:16, cw:cw + 64]
                    idxs = blk.T.reshape(-1).astype(np.int64)   # [1024]
                    rows = h1_pairs[base_row + idxs]            # [1024, 128]
                    g0 = ph * 32 + c * 8
                    g2[:, g0:g0 + 8, :] = \
                        rows.reshape(8, P, 2 * D).transpose(1, 0, 2)
            oh2b = oh2[b * P:(b + 1) * P]
            ghb = ghohm[b * P:(b + 1) * P]
            for ti in range(B):
                t = b * B + ti
                aggrT = np.zeros((D, P), np.float32)
                for cls in range(4):
                    phh, par = cls // 2, cls % 2
                    for j in range(4):
                        bc = ti * 16 + cls * 4 + j
                        gcol = phh * 32 + par * 16 + ti * 4 + j
                        oh = oh2b[:, bc * P:(bc + 1) * P]
                        aggrT += g2[:, gcol, par * D:(par + 1) * D].T @ oh
                aggrT = bf(aggrT)
                pre = (wts[:, 128:192].T @ aggrT
                       + wts[:, 192:256].T @ h1T[:, t * P:(t + 1) * P]
                       + f32(m["bias"])[:, 1][:, None])
                h2T = bf(np.maximum(pre, 0.0))
                ghoh = ghb[:, ti * G:(ti + 1) * G]
                pooled += bf(h2T) @ ghoh

    m0 = in_maps[0]
    poolA = np.concatenate([pooled, np.ones((1, G), np.float32)], axis=0)
    logits = poolA.T @ f32(m0["fcw"])
    mx = logits.max(axis=1, keepdims=True)
    lse = np.log(np.exp(logits - mx).sum(axis=1, keepdims=True))
    return logits - mx - lse


# ---------------------------------------------------------------------------
# Bass kernel builder
# ---------------------------------------------------------------------------

def build_kernel(T, cfg=CFG):
    import concourse.bass as bass
    import concourse.bacc as bacc
    import concourse.tile as tile
    import concourse.mybir as mybir
    from concourse.masks import make_identity

    F32 = mybir.dt.float32
    BF = mybir.dt.bfloat16
    F8 = mybir.dt.float8e4
    I16 = mybir.dt.int16
    AF = mybir.ActivationFunctionType
    OP = mybir.AluOpType

    N, D, H, O, G = cfg["N"], cfg["E"], cfg["D"], cfg["H"], cfg["G"]
    N, E, D, H, O, G = (cfg["N"], cfg["E"], cfg["D"], cfg["H"], cfg["O"],
                        cfg["G"])
    NC, B, P = cfg["NCORES"], cfg["B"], cfg["P"]
    NB = T // B
    NPH2 = 2
    PH_ROWS = NC * T * P // NPH2
    H1ROWS = NC * T * P

    nc = bacc.Bacc("TRN2", target_bir_lowering=False, debug=False,
                   num_devices=NC, num_swdge_queues=4)

    oh1_t = nc.dram_tensor("oh1", [NB * P, 64 * P], BF, kind="ExternalInput")
    oh2_t = nc.dram_tensor("oh2", [NB * P, 64 * P], BF, kind="ExternalInput")
    ghoh_t = nc.dram_tensor("ghoh", [NB * P, B * G], BF,
                            kind="ExternalInput")
    tab1_t = nc.dram_tensor("tab1", [NB * P, 64 * D], BF,
                            kind="ExternalInput")
    idxL2_t = nc.dram_tensor("idxL2", [P, NB * 8 * 64], I16,
                             kind="ExternalInput")
    xT_t = nc.dram_tensor("xT", [D, T * P], BF, kind="ExternalInput")
    wts_t = nc.dram_tensor("wts", [D, 256], BF, kind="ExternalInput")
    bias_t = nc.dram_tensor("bias", [D, 2], F32, kind="ExternalInput")
    fcw_t = nc.dram_tensor("fcw", [D + 1, O], F32, kind="ExternalInput")
    out_t = nc.dram_tensor("out", [G, O], F32, kind="ExternalOutput")

    with tile.TileContext(nc, num_cores=NC) as tc:
        with (
            tc.tile_pool(name="cst", bufs=1) as cst,
            tc.tile_pool(name="slab", bufs=1) as slab,
            tc.tile_pool(name="g1p", bufs=2) as g1p,
            tc.tile_pool(name="g2p", bufs=2) as g2p,
            tc.tile_pool(name="idxp", bufs=2) as idxp,
            tc.tile_pool(name="ohbp", bufs=3) as ohbp,
            tc.tile_pool(name="ghp", bufs=2) as ghp,
            tc.tile_pool(name="agp", bufs=2) as agp,
            tc.tile_pool(name="rowp", bufs=3) as rowp,
            tc.tile_pool(name="smallp", bufs=2) as smallp,
            tc.tile_pool(name="ps_ag", bufs=2, space="PSUM") as ps_ag,
            tc.tile_pool(name="ps_h", bufs=2, space="PSUM") as ps_h,
            tc.tile_pool(name="ps_tr", bufs=2, space="PSUM") as ps_tr,
            tc.tile_pool(name="ps_pool", bufs=1, space="PSUM") as ps_pool,
            tc.tile_pool(name="dram", bufs=1, space="DRAM") as dram,
        ):
            # ---- constants -------------------------------------------------
            wts = cst.tile([D, 256], BF)
            nc.sync.dma_start(wts[:], wts_t[:])
            bias = cst.tile([D, 2], F32)
            nc.sync.dma_start(bias[:], bias_t[:])
            fcw = cst.tile([D + 1, O], F32)
            nc.sync.dma_start(fcw[:], fcw_t[:])
            xT = slab.tile([D, T * P], BF)
            nc.sync.dma_start(xT[:], xT_t[:])
            identb = cst.tile([P, P], BF)
            make_identity(nc, identb[:])
            # preload ACT Exp/Ln tables so the tail doesn't stall on them
            warm = cst.tile([1, 4], F32)
            nc.gpsimd.memset(warm[:], 1.0)
            warm2 = cst.tile([1, 4], F32)
            nc.scalar.activation(out=warm2[:], in_=warm[:], func=AF.Exp)
            nc.scalar.activation(out=warm2[:], in_=warm[:], func=AF.Ln)

            h1T = slab.tile([D, T * P], BF)

            h1_local = dram.tile([T * P, D], BF)
            h1_full = dram.tile([H1ROWS // 2, 2 * D], BF, addr_space="Shared")
            pool_in = dram.tile([D, G], F32)
            pool_out = dram.tile([D, G], F32, addr_space="Shared")

            psum_pool = ps_pool.tile([D, G], F32)

            def tile_tail(lyr, t, aggrT, ti, ghoh_sb):
                """weights + relu (+transpose/pool for L2)."""
                wcol = 0 if lyr == 0 else 128
                psum_hT = ps_h.tile([D, P], F32, tag="hT")
                nc.tensor.matmul(
                    psum_hT[:], lhsT=wts[:, wcol:wcol + 64],
                    rhs=aggrT[:, ti * P:(ti + 1) * P],
                    start=True, stop=False)
                rhs_self = (xT if lyr == 0 else h1T)
                nc.tensor.matmul(
                    psum_hT[:], lhsT=wts[:, wcol + 64:wcol + 128],
                    rhs=rhs_self[:, t * P:(t + 1) * P],
                    start=False, stop=True)
                if lyr == 0:
                    nc.scalar.activation(
                        out=h1T[:, t * P:(t + 1) * P], in_=psum_hT[:],
                        func=AF.Relu, bias=bias[:, 0:1], scale=1.0)
                    psum_tr = ps_tr.tile([P, D], BF, tag="tr")
                    nc.tensor.transpose(
                        psum_tr[:], h1T[:, t * P:(t + 1) * P],
                        identb[:D, :D])
                    h1row = rowp.tile([P, D], BF, tag="row")
                    nc.scalar.activation(out=h1row[:], in_=psum_tr[:],
                                         func=AF.Copy, scale=1.0)
                    nc.sync.dma_start(
                        h1_local[:][t * P:(t + 1) * P, :], h1row[:])
                else:
                    h2T = smallp.tile([D, P], BF, tag="h2T")
                    nc.scalar.activation(
                        out=h2T[:], in_=psum_hT[:],
                        func=AF.Relu, bias=bias[:, 1:2], scale=1.0)
                    psum_tr = ps_tr.tile([P, D], BF, tag="tr")
                    nc.tensor.transpose(psum_tr[:], h2T[:], identb[:D, :D])
                    h2row = rowp.tile([P, D], BF, tag="row")
                    nc.scalar.activation(out=h2row[:], in_=psum_tr[:],
                                         func=AF.Copy, scale=1.0)
                    nc.tensor.matmul(
                        psum_pool[:], lhsT=h2row[:],
                        rhs=ghoh_sb[:, ti * G:(ti + 1) * G],
                        start=(t == 0), stop=(t == T - 1),
                        skip_group_check=True,
                    )

            # ================= layer 1 =====================================
            for b in range(NB):
                g1 = g1p.tile([P, 64 * D], BF, tag="g1")
                nc.sync.dma_start(g1[:], tab1_t[b * P:(b + 1) * P, :])
                oh_sb = ohbp.tile([P, 64 * P], BF, tag="ohblk")
                nc.sync.dma_start(oh_sb[:], oh1_t[b * P:(b + 1) * P, :])
                psum_bank = ps_ag.tile([D, B * P], F32, tag="aggr")
                for ti in range(B):
                    t = b * B + ti
                    for ch in range(16):
                        lane = ti * 16 + ch
                        nc.tensor.matmul(
                            psum_bank[:, ti * P:(ti + 1) * P],
                            lhsT=g1[:, lane * D:(lane + 1) * D],
                            rhs=oh_sb[:, lane * P:(lane + 1) * P],
                            start=(ch == 0), stop=(ch == 15),
                        )
                aggrT = agp.tile([D, B * P], BF, tag="aggrT")
                nc.scalar.activation(out=aggrT[:], in_=psum_bank[:],
                                     func=AF.Copy, scale=1.0)
                for ti in range(B):
                    tile_tail(0, b * B + ti, aggrT, ti, None)

            # ================= AllGather h1 ================================
            nc.gpsimd.collective_compute(
                "AllGather", mybir.AluOpType.bypass,
                replica_groups=[list(range(NC))],
                ins=[h1_local.opt()], outs=[h1_full.opt()],
            )

            # ================= layer 2 =====================================
            for b in range(NB):
                idx_sb = idxp.tile([P, 8 * 64], I16, tag="idx")
                nc.sync.dma_start(
                    idx_sb[:], idxL2_t[:, b * 8 * 64:(b + 1) * 8 * 64])
                g2 = g2p.tile([P, 64, 2 * D], BF, tag="g2")
                for ph in range(NPH2):
                    lo = ph * (PH_ROWS // 2)
                    in_ap = h1_full[:][lo:lo + PH_ROWS // 2, :]
                    for c in range(4):
                        g0 = ph * 32 + c * 8
                        nc.gpsimd.dma_gather(
                            out_ap=g2[:, g0:g0 + 8, :],
                            in_ap=in_ap,
                            idxs_ap=idx_sb[:, (ph * 4 + c) * 64:
                                           (ph * 4 + c + 1) * 64],
                            num_idxs=1024,
                            num_idxs_reg=1024,
                            elem_size=2 * D,
                            single_packet=True,
                            queue_num=(b * 8 + ph * 4 + c) % 4,
                        )
                oh_sb = ohbp.tile([P, 64 * P], BF, tag="ohblk")
                nc.sync.dma_start(oh_sb[:], oh2_t[b * P:(b + 1) * P, :])
                ghoh_sb = ghp.tile([P, B * G], BF, tag="gh")
                nc.sync.dma_start(ghoh_sb[:], ghoh_t[b * P:(b + 1) * P, :])
                psum_bank = ps_ag.tile([D, B * P], F32, tag="aggr")
                for ti in range(B):
                    t = b * B + ti
                    nmm = 0
                    for cls in range(4):
                        phh, par = cls // 2, cls % 2
                        for j in range(4):
                            bc = ti * 16 + cls * 4 + j
                            gcol = phh * 32 + par * 16 + ti * 4 + j
                            nc.tensor.matmul(
                                psum_bank[:, ti * P:(ti + 1) * P],
                                lhsT=g2[:, gcol, par * D:(par + 1) * D],
                                rhs=oh_sb[:, bc * P:(bc + 1) * P],
                                start=(nmm == 0), stop=(nmm == 15),
                            )
                            nmm += 1
                aggrT = agp.tile([D, B * P], BF, tag="aggrT")
                nc.scalar.activation(out=aggrT[:], in_=psum_bank[:],
                                     func=AF.Copy, scale=1.0)
                for ti in range(B):
                    tile_tail(1, b * B + ti, aggrT, ti, ghoh_sb)

            # ---- pooled AllReduce + logits + log_softmax -------------------
            pooled_sb = slab.tile([D + 1, G], F32)
            nc.gpsimd.memset(pooled_sb[D:D + 1, :], 1.0)
            nc.vector.tensor_copy(out=pooled_sb[:D, :], in_=psum_pool[:])
            nc.sync.dma_start(pool_in[:], pooled_sb[:D, :])
            nc.gpsimd.collective_compute(
                "AllReduce", mybir.AluOpType.add,
                replica_groups=[list(range(NC))],
                ins=[pool_in.opt()], outs=[pool_out.opt()],
            )
            nc.sync.dma_start(pooled_sb[:D, :], pool_out[:])
            for gt in range(-(-G // P)):
                gsz = min(P, G - gt * P)
                psum_lg = ps_pool.tile([gsz, O], F32, tag="lg")
                nc.tensor.matmul(
                    psum_lg[:], lhsT=pooled_sb[:, gt * P:gt * P + gsz],
                    rhs=fcw[:], start=True, stop=True)
                mx = smallp.tile([gsz, 1], F32, tag="mx")
                nc.vector.tensor_reduce(
                    out=mx[:], in_=psum_lg[:], axis=mybir.AxisListType.X,
                    op=OP.max)
                nmx = smallp.tile([gsz, 1], F32, tag="nmx")
                nc.vector.tensor_scalar(
                    out=nmx[:], in0=mx[:], scalar1=-1.0, scalar2=None,
                    op0=OP.mult)
                ex = smallp.tile([gsz, O], F32, tag="ex")
                sumexp = smallp.tile([gsz, 1], F32, tag="se")
                nc.scalar.activation(
                    out=ex[:], in_=psum_lg[:], func=AF.Exp,
                    bias=nmx[:], scale=1.0, accum_out=sumexp[:])
                lse = smallp.tile([gsz, 1], F32, tag="lse")
                nc.scalar.activation(
                    out=lse[:], in_=sumexp[:], func=AF.Ln)
                res = smallp.tile([gsz, O], F32, tag="res")
                nc.vector.tensor_scalar(
                    out=res[:], in0=psum_lg[:], scalar1=nmx[:],
                    scalar2=lse[:], op0=OP.add, op1=OP.subtract)
                nc.sync.dma_start(out_t[gt * P:gt * P + gsz, :], res[:])

    nc.compile()
    return nc


# ---------------------------------------------------------------------------
# Entry point
# ---------------------------------------------------------------------------

def kernel(x, W1l, b1, W1r, W2l, b2, W2r, fcW, fcb, edge_index, batch,
           _cfg=None, _collect=None):
    cfg = _cfg or CFG
    in_maps, info = preprocess(x, W1l, b1, W1r, W2l, b2, W2r, fcW, fcb,
                               edge_index, batch, cfg)
    key = (info["T"], tuple(sorted(cfg.items())))
    if key not in _BUILD_CACHE:
        _BUILD_CACHE[key] = build_kernel(info["T"], cfg)
    nc = _BUILD_CACHE[key]

    from concourse.bass_utils import run_bass_kernel_spmd
    res = run_bass_kernel_spmd(
        nc, in_maps, core_ids=list(range(cfg["NCORES"])),
        **(_collect or {}))
    if _collect is not None:
        kernel._last_result = res
    return res.results[0]["out"]


if __name__ == "__main__":
    pass
